# revision 4
# baseline (speedup 1.0000x reference)
"""HQQ 4-bit dequant + linear: y = x @ dequant(W_q) + bias on 8 TRN2 cores.

Column-parallel: W/scale/zero/bias sharded along out_features, x replicated.
Out features padded 11008 -> 11264 = 8 * 1408 so every core runs the same
program; pad columns use scale=0 so they contribute nothing and are dropped
on the host after gather.

Per-core device program (all matmuls in float32r — 1 cycle/row on PE):
  for kb in 4 k-blocks (1024 k-rows each; kb 0/1 = high nibbles of byte rows
                        0..2047, kb 2/3 = low nibbles of the same rows):
    dequant W-block: per 128-row chunk, DVE unpack (shift/and) then fused
      (q - zero) * scale tensor_scalar per 64-col group (per-partition scalars)
    for mb in 8 token-blocks (256 tokens):
      one DMA loads xT[kb rows, mb tokens] as [128, 8, 256]
      accumulate psum[m2] over the 8 chunks (3 matmuls of N=512/512/384 each)
      y_acc[m] (+)= psum (+ bias on kb 0); DMA out after the last kb
"""
import numpy as np

IN_F = 4096
OUT_F = 11008
TOKENS = 2048
GROUP = 64
NCORES = 8
OUT_PAD = 11264          # 8 * 1408
C = OUT_PAD // NCORES    # 1408 cols per core
NG = C // GROUP          # 22 groups per row per core
KB = 4                   # k-blocks
KC = 8                   # chunks per k-block (128 rows each)
MB = 8                   # token blocks of 256
M2 = 2                   # 128-token tiles per token block
NSL = [(0, 512), (512, 1024), (1024, 1408)]

_TRACE = False
_LAST_RESULTS = None


def _build_nc(split=True):
    import concourse.bass as bass
    import concourse.tile as tile
    from concourse import mybir
    from waitsplit import split_excess_waits

    f32 = mybir.dt.float32
    f32r = mybir.dt.float32r
    u8 = mybir.dt.uint8
    SHR = mybir.AluOpType.logical_shift_right
    AND = mybir.AluOpType.bitwise_and
    SUB = mybir.AluOpType.subtract
    MUL = mybir.AluOpType.mult
    ADD = mybir.AluOpType.add

    nc = bass.Bass("TRN2", target_bir_lowering=False)
    xT = nc.dram_tensor("xT", [IN_F, TOKENS], f32, kind="ExternalInput")
    Bq = nc.dram_tensor("Bq", [IN_F // 2, C], u8, kind="ExternalInput")
    prm = nc.dram_tensor("prm", [IN_F, 2, NG], f32, kind="ExternalInput")
    bias = nc.dram_tensor("bias", [C], f32, kind="ExternalInput")
    y = nc.dram_tensor("y", [TOKENS, C], f32, kind="ExternalOutput")

    with tile.TileContext(nc) as tc:
        with (
            tc.tile_pool(name="singles", bufs=1) as singles,
            tc.tile_pool(name="yacc", bufs=1) as yacc_pool,
            tc.tile_pool(name="wpool", bufs=10) as wpool,
            tc.tile_pool(name="bytes", bufs=3) as bpool,
            tc.tile_pool(name="nibs", bufs=3) as npool,
            tc.tile_pool(name="xts", bufs=2) as xpool,
            tc.tile_pool(name="psum", bufs=2, space="PSUM") as ppool,
        ):
            # bias replicated across partitions
            bias_t = singles.tile([128, C], f32)
            bias_ap = bias[:]
            nc.sync.dma_start(
                out=bias_t,
                in_=bass.AP(
                    tensor=bias_ap.tensor, offset=bias_ap.offset,
                    ap=[[0, 128]] + list(bias_ap.ap),
                ),
            )
            # dequant params: [128, kc_global, {zero,scale}, group]
            prm_raw = singles.tile([128, 32, 2, NG], f32)
            nc.sync.dma_start(
                out=prm_raw,
                in_=prm.rearrange("(c p) t g -> p c t g", p=128),
            )
            # route through DVE so dequant ops have same-engine deps only
            prm2 = singles.tile([128, 32, 2, NG], f32)
            nc.vector.tensor_copy(prm2[:, :, :, :], prm_raw[:, :, :, :])

            y_acc = []
            for m in range(16):
                ya = yacc_pool.tile([128, C], f32, name=f"ya{m}")
                y_acc.append(ya)

            for kb in range(KB):
                brow0 = (kb % 2) * 1024
                hi = kb < 2
                w_tiles = []
                for kc in range(KC):
                    kcg = kb * KC + kc  # global k-chunk index
                    bt = bpool.tile([128, C], u8, tag="bt")
                    nc.sync.dma_start(
                        out=bt, in_=Bq[brow0 + kc * 128 : brow0 + (kc + 1) * 128, :]
                    )
                    nib = npool.tile([128, C], u8, tag="nib")
                    if hi:
                        nc.vector.tensor_scalar(nib[:, :], bt[:, :], 4, None, SHR)
                    else:
                        nc.vector.tensor_scalar(nib[:, :], bt[:, :], 15, None, AND)
                    wt = wpool.tile([128, C], f32r, tag="wt")
                    for g in range(NG):
                        sl = slice(g * GROUP, (g + 1) * GROUP)
                        nc.vector.tensor_scalar(
                            wt[:, sl],
                            nib[:, sl],
                            prm2[:, kcg, 0, g : g + 1],
                            prm2[:, kcg, 1, g : g + 1],
                            SUB,
                            MUL,
                        )
                    w_tiles.append(wt)

                for mb in range(MB):
                    t0 = mb * 256
                    xt = xpool.tile([128, KC, 256], f32r, tag="xt")
                    nc.sync.dma_start(
                        out=xt,
                        in_=xT[
                            kb * 1024 : (kb + 1) * 1024, t0 : t0 + 256
                        ].rearrange("(kc p) t -> p kc t", p=128).bitcast(f32r),
                    )
                    ps = []
                    for m2 in range(M2):
                        pt = ppool.tile([128, C], f32, tag="ps")
                        ps.append(pt)
                    for kc in range(KC):
                        for m2 in range(M2):
                            lhsT = xt[:, kc, m2 * 128 : (m2 + 1) * 128]
                            for (n0, n1) in NSL:
                                nc.tensor.matmul(
                                    ps[m2][:, n0:n1],
                                    lhsT,
                                    w_tiles[kc][:, n0:n1],
                                    start=(kc == 0),
                                    stop=(kc == KC - 1),
                                )
                    for m2 in range(M2):
                        m = mb * 2 + m2
                        if kb == 0:
                            nc.vector.tensor_tensor(
                                y_acc[m][:, :], ps[m2][:, :], bias_t[:, :], ADD
                            )
                        else:
                            nc.vector.tensor_tensor(
                                y_acc[m][:, :], ps[m2][:, :], y_acc[m][:, :], ADD
                            )
                        if kb == KB - 1:
                            nc.sync.dma_start(
                                out=y[m * 128 : (m + 1) * 128, :], in_=y_acc[m]
                            )

    if split:
        split_excess_waits(nc)
    return nc


def _prepare_in_maps(x, W_q, scale, zero, bias):
    x = np.asarray(x, dtype=np.float32)
    W_q = np.asarray(W_q)
    scale = np.asarray(scale, dtype=np.float32)
    zero = np.asarray(zero, dtype=np.float32)
    bias = np.asarray(bias, dtype=np.float32)

    xT = np.ascontiguousarray(x.T)  # [IN_F, TOKENS]
    B = (W_q.astype(np.uint32) & 0xFF).astype(np.uint8).reshape(IN_F // 2, OUT_F)
    B_pad = np.zeros((IN_F // 2, OUT_PAD), dtype=np.uint8)
    B_pad[:, :OUT_F] = B
    sc = np.zeros((IN_F, OUT_PAD // GROUP), dtype=np.float32)
    zr = np.zeros((IN_F, OUT_PAD // GROUP), dtype=np.float32)
    sc[:, : OUT_F // GROUP] = scale.reshape(IN_F, OUT_F // GROUP)
    zr[:, : OUT_F // GROUP] = zero.reshape(IN_F, OUT_F // GROUP)
    bias_pad = np.zeros(OUT_PAD, dtype=np.float32)
    bias_pad[:OUT_F] = bias

    in_maps = []
    for s in range(NCORES):
        c0, c1 = s * C, (s + 1) * C
        g0, g1 = c0 // GROUP, c1 // GROUP
        prm_s = np.ascontiguousarray(
            np.stack([zr[:, g0:g1], sc[:, g0:g1]], axis=1)
        )  # [IN_F, 2, NG]
        in_maps.append(
            dict(
                xT=xT,
                Bq=np.ascontiguousarray(B_pad[:, c0:c1]),
                prm=prm_s,
                bias=np.ascontiguousarray(bias_pad[c0:c1]),
            )
        )
    return in_maps


def kernel(x, W_q, scale, zero, bias):
    global _LAST_RESULTS
    from concourse.bass_utils import run_bass_kernel_spmd

    in_maps = _prepare_in_maps(x, W_q, scale, zero, bias)
    nc = _build_nc()
    res = run_bass_kernel_spmd(
        nc, in_maps, core_ids=list(range(NCORES)), trace=_TRACE
    )
    _LAST_RESULTS = res
    y_full = np.concatenate([res.results[s]["y"] for s in range(NCORES)], axis=1)
    return np.ascontiguousarray(y_full[:, :OUT_F])
